# revision 1
# baseline (speedup 1.0000x reference)
"""GCN encoder (2-layer) on 8 Trainium2 NeuronCores.

Math (per layer, matching the reference):
    out[d] = dis[d] * sum_{e: dst_e=d} dis[src_e] * h[src_e]  + b
with h = x @ W, dis = deg^-1/2 over src-with-self-loops. dis factors are
folded host-side: xT is pre-scaled by dis (layer-1 operand), layer-1's
output scaling uses dis^2 (post relu identity: dis*relu(z) = relu(dis*z)),
layer 2 applies dis at the end.

This environment executes roughly one engine instruction per ~55us with no
cross-engine overlap, so the design minimizes instruction count:
  - edges per dst-window (128 dsts) are gathered in [rank, slot] order so
    token k*128+p is the k-th in-edge of window-slot p; one wide
    tensor_reduce over the rank axis aggregates a whole window.
  - dma_gather with single_packet=False allows ~8192 indices/instruction
    (single_packet=True hangs above ~1024).
  - rank padding points at injected all-zero rows: every core ships 6251
    rows (row 6250 zeroed), so zero rows exist in both the lo ([0,32768))
    and hi ([32768,50008)) gather bases of the int16-index split.
Sharding: nodes row-sharded 6250/core, edges partitioned by dst core,
weights replicated, AllGather between layers.
"""
import os
import numpy as np

N, E = 50000, 1600000
FIN, FHID, FOUT = 256, 128, 64
NCORES = 8
NPC = N // NCORES          # 6250
NPC2 = NPC + 1             # 6251 rows shipped per core (last = zeros)
NFULL = NCORES * NPC2      # 50008
NW = (NPC + 127) // 128    # 49 windows
NPAD = NW * 128            # 6272
HALF = 32768               # int16 gather base split
ZLO = 6250                 # zero row inside lo base (core 0 pad row)
ZHI = 5 * NPC2 + NPC - HALF  # core 5 pad row, hi-base-local index
MAXRANKS = 64              # ranks per gather instruction (8192 idxs)

_CACHE = {}
LAST_RESULTS = None


def _host_prep(x, edge_index, W1, b1, W2, b2):
    x = np.asarray(x, dtype=np.float32)
    ei = np.asarray(edge_index)
    W1 = np.asarray(W1, dtype=np.float32)
    W2 = np.asarray(W2, dtype=np.float32)
    b1 = np.asarray(b1, dtype=np.float32)
    b2 = np.asarray(b2, dtype=np.float32)

    loops = np.arange(N, dtype=np.int64)
    src = np.concatenate([ei[0].astype(np.int64), loops])
    dst = np.concatenate([ei[1].astype(np.int64), loops])

    deg = np.bincount(src, minlength=N).astype(np.float32)
    dis = np.power(deg, np.float32(-0.5), dtype=np.float32)
    dis[deg == 0] = 0.0

    # padded gather row of each source node
    r_all = (src // NPC) * NPC2 + (src % NPC)
    s_all = (r_all >= HALF).astype(np.int64)  # 0 = lo stream, 1 = hi

    core = dst // NPC
    order = np.argsort(dst, kind="stable")
    r_s, dst_s, s_s = r_all[order], dst[order], s_all[order]
    cb = np.searchsorted(dst_s, np.arange(NCORES + 1) * NPC)

    # per-core rank assignment within (dst, stream)
    percore = []
    KLO = np.zeros((NCORES, NW), np.int64)
    KHI = np.zeros((NCORES, NW), np.int64)
    for c in range(NCORES):
        sl = slice(cb[c], cb[c + 1])
        r_c = r_s[sl]
        d_c = dst_s[sl] - c * NPC
        s_c = s_s[sl]
        key = d_c * 2 + s_c
        o2 = np.argsort(key, kind="stable")
        key_o = key[o2]
        first = np.searchsorted(key_o, key_o, side="left")
        rank = np.arange(len(key_o)) - first
        d_o, s_o, r_o = d_c[o2], s_c[o2], r_c[o2]
        w_o, p_o = d_o // 128, d_o % 128
        np.maximum.at(KLO[c], w_o[s_o == 0], rank[s_o == 0] + 1)
        np.maximum.at(KHI[c], w_o[s_o == 1], rank[s_o == 1] + 1)
        percore.append((w_o, p_o, s_o, rank, r_o))

    KLOm = KLO.max(axis=0)  # [NW]
    KHIm = KHI.max(axis=0)
    # pad window pairs (2w, 2w+1) to equal total ranks so one 4D-AP
    # tensor_reduce can aggregate both windows at once
    Kt = KLOm + KHIm
    for i in range(0, NW - 1, 2):
        kp = max(Kt[i], Kt[i + 1])
        KHIm[i] += kp - Kt[i]
        KHIm[i + 1] += kp - Kt[i + 1]
    K = KLOm + KHIm
    # flat token-position offsets: window w = [lo ranks][hi ranks]
    woff = np.zeros(NW + 1, np.int64)
    woff[1:] = np.cumsum(K) * 128
    total_tok = int(woff[-1])

    in_maps = []
    for c in range(NCORES):
        w_o, p_o, s_o, rank, r_o = percore[c]
        gidx = np.empty(total_tok, np.int16)
        for w in range(NW):
            gidx[woff[w]:woff[w] + KLOm[w] * 128] = ZLO
            gidx[woff[w] + KLOm[w] * 128:woff[w + 1]] = ZHI
        pos = woff[w_o] + (rank + np.where(s_o == 1, KLOm[w_o], 0)) * 128 + p_o
        gidx[pos] = np.where(s_o == 1, r_o - HALF, r_o).astype(np.int16)
        gidx_t = np.tile(gidx.reshape(-1, 16).T, (8, 1))  # [128, total_tok//16]

        dis_l = dis[c * NPC:(c + 1) * NPC]
        dis_pad = np.zeros(NPAD, np.float32)
        dis_pad[:NPC] = dis_l
        dis_col = np.ascontiguousarray(dis_pad.reshape(NW, 128).T)  # [128, NW]
        dis2_col = dis_col * dis_col
        # Bstt[p, w*128+f] = dis[w*128+p] * b1[f]
        Bstt = (dis_col.T[:, :, None] * b1[None, None, :]).transpose(1, 0, 2)
        Bstt = np.ascontiguousarray(Bstt.reshape(128, NW * FHID))

        xT = np.zeros((FIN, NPAD), np.float32)
        xT[:, :NPC] = (x[c * NPC:(c + 1) * NPC] * dis_l[:, None]).T

        in_maps.append({
            "gidx": np.ascontiguousarray(gidx_t),
            "xT": xT,
            "W1": W1, "W2": W2,
            "dis2c": dis2_col, "disc": dis_col,
            "Bstt": Bstt,
            "b2b": np.tile(b2, (128, 1)),
            "ident": np.eye(128, dtype=np.float32),
        })
    return in_maps, (KLOm, KHIm, bool(not b1.any()))


def _build(Kinfo):
    import concourse.bacc as bacc
    import concourse.mybir as mybir
    import concourse.tile as tile

    KLOm, KHIm, B1ZERO = Kinfo
    K = KLOm + KHIm
    maxK = max(int(K[i]) * (1 if i + 1 >= NW else 2)
               for i in range(0, NW, 2))
    total_tok = int(K.sum()) * 128

    PHASES = os.environ.get("GCN_PHASES", "full")
    REPEAT = int(os.environ.get("GCN_REPEAT", "1"))

    dt = mybir.dt
    ALU = mybir.AluOpType

    nc = bacc.Bacc("TRN2", target_bir_lowering=False, debug=False,
                   num_devices=NCORES)

    gidx_d = nc.dram_tensor("gidx", [128, total_tok // 16], dt.int16, kind="ExternalInput")
    xT_d = nc.dram_tensor("xT", [FIN, NPAD], dt.float32, kind="ExternalInput")
    W1_d = nc.dram_tensor("W1", [FIN, FHID], dt.float32, kind="ExternalInput")
    W2_d = nc.dram_tensor("W2", [FHID, FOUT], dt.float32, kind="ExternalInput")
    dis2_d = nc.dram_tensor("dis2c", [128, NW], dt.float32, kind="ExternalInput")
    dis_d = nc.dram_tensor("disc", [128, NW], dt.float32, kind="ExternalInput")
    Bstt_d = nc.dram_tensor("Bstt", [128, NW * FHID], dt.float32, kind="ExternalInput")
    b2b_d = nc.dram_tensor("b2b", [128, FOUT], dt.float32, kind="ExternalInput")
    ident_d = nc.dram_tensor("ident", [128, 128], dt.float32, kind="ExternalInput")
    out_d = nc.dram_tensor("out", [NPC, FOUT], dt.float32, kind="ExternalOutput")

    t1_local = nc.dram_tensor("t1_local", [NPC2, FHID], dt.float32)
    t1_full = nc.dram_tensor("t1_full", [NFULL, FHID], dt.float32, addr_space="Shared")
    t2_local = nc.dram_tensor("t2_local", [NPC2, FOUT], dt.float32)
    t2_full = nc.dram_tensor("t2_full", [NFULL, FOUT], dt.float32, addr_space="Shared")

    with tile.TileContext(nc) as tc:
        with (
            tc.tile_pool(name="consts", bufs=1) as cp,
            tc.tile_pool(name="work", bufs=1) as wp,
            tc.tile_pool(name="psum", bufs=1, space="PSUM") as pp,
        ):
            ident_t = cp.tile([128, 128], dt.float32, tag="ident")
            nc.sync.dma_start(ident_t[:], ident_d[:, :])
            w1_t = cp.tile([128, 2, FHID], dt.float32, tag="w1")
            nc.sync.dma_start(w1_t[:, 0, :], W1_d[0:128, :])
            nc.sync.dma_start(w1_t[:, 1, :], W1_d[128:256, :])
            w2_t = cp.tile([FHID, FOUT], dt.float32, tag="w2")
            nc.sync.dma_start(w2_t[:], W2_d[:, :])
            dis2_t = cp.tile([128, NW], dt.float32, tag="dis2")
            nc.sync.dma_start(dis2_t[:], dis2_d[:, :])
            dis_t = cp.tile([128, NW], dt.float32, tag="dis")
            nc.sync.dma_start(dis_t[:], dis_d[:, :])
            if not B1ZERO:
                Bstt_t = cp.tile([128, NW * FHID], dt.float32, tag="Bstt")
                nc.sync.dma_start(Bstt_t[:], Bstt_d[:, :])
            b2b_t = cp.tile([128, FOUT], dt.float32, tag="b2b")
            nc.sync.dma_start(b2b_t[:], b2b_d[:, :])
            gidx_t = cp.tile([128, total_tok // 16], dt.int16, tag="gidx")
            nc.sync.dma_start(gidx_t[:], gidx_d[:, :])
            zrow = cp.tile([128, FHID], dt.float32, tag="zrow")
            nc.vector.memset(zrow[:], 0.0)

            # one shared gpsimd register per distinct gather count: avoids a
            # RegisterMove instruction (~55us here) per dma_gather
            counts = set()
            for w in range(NW):
                for nk in (int(KLOm[w]), int(KHIm[w])):
                    for k0 in range(0, nk, MAXRANKS):
                        counts.add(min(MAXRANKS, nk - k0) * 128)
            nidx_regs = {cnt: nc.gpsimd.to_reg(cnt) for cnt in sorted(counts)}

            for _rep in range(REPEAT):
                # ---- phase B: t1_local = (dis*x) @ W1 ----
                with tc.tile_pool(name="phaseB", bufs=1) as pb:
                    xT_t = pb.tile([128, 2, NPAD], dt.float32, tag="xT")
                    nc.sync.dma_start(xT_t[:, 0, :], xT_d[0:128, :])
                    nc.sync.dma_start(xT_t[:, 1, :], xT_d[128:256, :])
                    evB = pb.tile([128, 8, FHID], dt.float32, tag="evB")
                    psB = pp.tile([128, 8, FHID], dt.float32, tag="pB")
                    for w in range(NW):
                        sl = psB[:, w % 8, :]
                        for kc in range(2):
                            nc.tensor.matmul(
                                sl, xT_t[:, kc, w * 128:w * 128 + 128],
                                w1_t[:, kc, :], start=(kc == 0), stop=(kc == 1))
                        if w % 8 == 7:
                            nc.vector.tensor_copy(evB[:], psB[:])
                        if w == 48:
                            nc.vector.tensor_copy(evB[:, 0, :], sl)
                        if w % 8 == 7:
                            nc.sync.dma_start(
                                t1_local[(w - 7) * 128:(w + 1) * 128, :]
                                .rearrange("(a p) f -> p a f", p=128),
                                evB[:])
                    # window 48 (106 rows)
                    nc.sync.dma_start(t1_local[48 * 128:NPC, :],
                                      evB[0:106, 0, :])
                    nc.sync.dma_start(t1_local[NPC:NPC2, :], zrow[0:1, :])

                nc.gpsimd.collective_compute(
                    "AllGather", mybir.AluOpType.bypass,
                    replica_groups=[list(range(NCORES))],
                    ins=[t1_local[:, :]], outs=[t1_full[:, :]],
                )

                if PHASES == "B":
                    ot = wp.tile([128, FOUT], dt.float32, tag="o")
                    nc.vector.memset(ot[:], 0.0)
                    for w in range(NW):
                        rows = min(128, NPC - w * 128)
                        nc.sync.dma_start(out_d[w * 128:w * 128 + rows, :],
                                          ot[0:rows, :])
                    continue

                def gather_window(tok, w, src_full, feat, woff_w, dk=0):
                    """Emit gathers for window w into tok at rank offset dk."""
                    klo, khi = int(KLOm[w]), int(KHIm[w])
                    base_lo = src_full[0:HALF, :]
                    base_hi = src_full[HALF:NFULL, :]
                    segs = [(0, klo, base_lo), (klo, khi, base_hi)]
                    for seg0, nk, base in segs:
                        for k0 in range(0, nk, MAXRANKS):
                            kn = min(MAXRANKS, nk - k0)
                            c0 = (woff_w + (seg0 + k0) * 128) // 16
                            d0 = dk + seg0 + k0
                            nc.gpsimd.dma_gather(
                                tok[:, d0:d0 + kn, :], base,
                                gidx_t[:, c0:c0 + kn * 8],
                                num_idxs=kn * 128,
                                num_idxs_reg=nidx_regs[kn * 128],
                                elem_size=feat, single_packet=False)

                # ---- L1 pass 1: gather + reduce + scale into o1s_all ----
                with tc.tile_pool(name="L1", bufs=1) as l1:
                    tok = l1.tile([128, maxK, FHID], dt.float32, tag="tok1")
                    red = l1.tile([128, 2, FHID], dt.float32, tag="red")
                    o1s_all = l1.tile([128, NW, FHID], dt.float32, tag="o1sa")
                    o1T = l1.tile([128, 4, FHID], dt.float32, tag="o1T")
                    ev1 = l1.tile([128, 8, FOUT], dt.float32, tag="ev1")
                    pT = pp.tile([128, 4, 512], dt.float32, tag="pT")  # slice per bank
                    p2 = pp.tile([128, 8, FOUT], dt.float32, tag="p2")
                    woff_w = 0
                    for w0 in range(0, NW, 2):
                        pair = [w0] if w0 + 1 >= NW else [w0, w0 + 1]
                        kp = int(K[w0])
                        for j, w in enumerate(pair):
                            gather_window(tok, w, t1_full, FHID,
                                          woff_w, j * kp)
                            woff_w += int(K[w]) * 128
                        nc.vector.tensor_reduce(
                            red[:, 0:len(pair), :],
                            tok[:, 0:len(pair) * kp, :]
                            .rearrange("p (b k) f -> p b f k", b=len(pair)),
                            mybir.AxisListType.X, ALU.add)
                        for j, w in enumerate(pair):
                            # o1s = relu(dis^2*red + dis*b1)
                            if B1ZERO:
                                nc.vector.tensor_scalar(
                                    o1s_all[:, w, :], red[:, j, :],
                                    dis2_t[:, w:w + 1],
                                    0.0, ALU.mult, ALU.max)
                            else:
                                nc.vector.scalar_tensor_tensor(
                                    o1s_all[:, w, :], red[:, j, :],
                                    dis2_t[:, w:w + 1],
                                    Bstt_t[:, w * 128:(w + 1) * 128],
                                    ALU.mult, ALU.add)
                                nc.vector.tensor_scalar(
                                    o1s_all[:, w, :], o1s_all[:, w, :], 0.0,
                                    None, ALU.max)
                    # ---- L1 pass 2: transpose + @W2, batched ----
                    for w in range(NW):
                        nc.tensor.transpose(pT[:, w % 4, 0:FHID],
                                            o1s_all[:, w, :], ident_t[:])
                        if w % 4 == 3:
                            nc.vector.tensor_copy(o1T[:], pT[:, :, 0:FHID])
                        if w == 48:
                            nc.vector.tensor_copy(o1T[:, 0, :], pT[:, 0, 0:FHID])
                        if w % 4 == 3 or w == 48:
                            for w2 in range(w - (3 if w % 4 == 3 else 0), w + 1):
                                nc.tensor.matmul(p2[:, w2 % 8, :],
                                                 o1T[:, w2 % 4, :], w2_t[:],
                                                 start=True, stop=True)
                        if w % 8 == 7:
                            nc.vector.tensor_copy(ev1[:], p2[:])
                        if w == 48:
                            nc.vector.tensor_copy(ev1[:, 0, :], p2[:, 0, :])
                        if w % 8 == 7:
                            nc.sync.dma_start(
                                t2_local[(w - 7) * 128:(w + 1) * 128, :]
                                .rearrange("(a p) f -> p a f", p=128),
                                ev1[:])
                    nc.sync.dma_start(t2_local[48 * 128:NPC, :],
                                      ev1[0:106, 0, :])
                    nc.sync.dma_start(t2_local[NPC:NPC2, :], zrow[0:1, 0:FOUT])

                if PHASES == "B1":
                    ot = wp.tile([128, FOUT], dt.float32, tag="o")
                    nc.vector.memset(ot[:], 0.0)
                    for w in range(NW):
                        rows = min(128, NPC - w * 128)
                        nc.sync.dma_start(out_d[w * 128:w * 128 + rows, :],
                                          ot[0:rows, :])
                    continue

                nc.gpsimd.collective_compute(
                    "AllGather", mybir.AluOpType.bypass,
                    replica_groups=[list(range(NCORES))],
                    ins=[t2_local[:, :]], outs=[t2_full[:, :]],
                )

                # ---- L2 windows ----
                with tc.tile_pool(name="L2", bufs=1) as l2:
                    tok2 = l2.tile([128, maxK, FOUT], dt.float32, tag="tok2")
                    red2 = l2.tile([128, 2, FOUT], dt.float32, tag="red2")
                    ev2 = l2.tile([128, 8, FOUT], dt.float32, tag="ev2")
                    woff_w = 0
                    for w0 in range(0, NW, 2):
                        pair = [w0] if w0 + 1 >= NW else [w0, w0 + 1]
                        kp = int(K[w0])
                        for j, w in enumerate(pair):
                            gather_window(tok2, w, t2_full, FOUT,
                                          woff_w, j * kp)
                            woff_w += int(K[w]) * 128
                        nc.vector.tensor_reduce(
                            red2[:, 0:len(pair), :],
                            tok2[:, 0:len(pair) * kp, :]
                            .rearrange("p (b k) f -> p b f k", b=len(pair)),
                            mybir.AxisListType.X, ALU.add)
                        for j, w in enumerate(pair):
                            nc.vector.scalar_tensor_tensor(
                                ev2[:, w % 8, :], red2[:, j, :],
                                dis_t[:, w:w + 1],
                                b2b_t[:], ALU.mult, ALU.add)
                        w = pair[-1]
                        if w % 8 == 7:
                            nc.sync.dma_start(
                                out_d[(w - 7) * 128:(w + 1) * 128, :]
                                .rearrange("(a p) f -> p a f", p=128),
                                ev2[:])
                    nc.sync.dma_start(out_d[48 * 128:NPC, :], ev2[0:106, 0, :])

    nc.compile()
    return nc


def kernel(x, edge_index, W1, b1, W2, b2):
    global LAST_RESULTS
    from concourse.bass_utils import run_bass_kernel_spmd

    in_maps, Kinfo = _host_prep(x, edge_index, W1, b1, W2, b2)
    key = (Kinfo[0].tobytes(), Kinfo[1].tobytes(), Kinfo[2])
    if key not in _CACHE:
        _CACHE[key] = _build(Kinfo)
    nc = _CACHE[key]

    res = run_bass_kernel_spmd(nc, in_maps, list(range(NCORES)))
    LAST_RESULTS = res
    return np.concatenate([res.results[c]["out"] for c in range(NCORES)], axis=0)



# revision 10
# speedup vs baseline: 1.4517x; 1.4517x over previous
"""GCN encoder (2-layer) on 8 Trainium2 NeuronCores — instruction-minimal design.

This environment executes roughly one engine instruction per ~55-67us with no
cross-engine overlap, so the design minimizes instruction count:

  - f-major compute: h1T = W1^T @ xT with nodes as the matmul free dim
    (512 nodes/matmul -> 26 matmuls vs 98 node-major), DMA-transpose (xbar)
    converts f-major SBUF tiles to node-major DRAM gather tables (fp16).
  - transpose-mode dma_gather (fp16, elem=128) yields tokens in [feat, token]
    layout; one strided 4D-AP tensor_reduce aggregates a whole multi-window
    group; whole-layer scalar_tensor_tensor applies relu/deg scaling.
  - host-side node permutation: nodes are dealt to cores by sorted in-degree
    and slotted within a core to balance per-(group,stream) max rank, cutting
    gather padding tokens ~33%; host un-permutes the final output for free.
  - lo/hi gather-base split at a core boundary (5/3) keeps int16 indices
    valid while making each edge's stream invariant to within-core slotting.
  - group boundaries chosen by DP minimizing gathers + reduce overhead under
    the SBUF token-buffer cap.

Sharding: nodes dealt 6250/core (permuted), edges partitioned by dst core,
weights replicated, fp16 AllGather between layers.
"""
import os
import numpy as np

N, E = 50000, 1600000
FIN, FHID, FOUT = 256, 128, 64
NCORES = 8
NPC = N // NCORES          # 6250
NW = 49                    # windows per core
NPAD = NW * 128            # 6272
NFULL = NCORES * NPAD      # 50176
LOCORES = 5
LOROWS = LOCORES * NPAD    # 31360 rows in the lo gather base (< 32768)
ZLO = 6250                 # all-zero pad row inside lo base (core 0)
ZHI = 6250                 # core 5 pad row, hi-base-local
MAXIDX = 8192              # max indices per dma_gather instruction
TOKCAP = 30720             # token-buffer capacity (60KB fp16 per partition)

_CACHE = {}
LAST_RESULTS = None


def _plan_groups(Lw, Hw):
    """DP over sorted windows: pick group boundaries minimizing
    gathers + 3 per group (2 reduces + 1 add), under TOKCAP."""
    NWn = len(Lw)
    INF = 1 << 30
    best = [INF] * (NWn + 1)
    prev = [0] * (NWn + 1)
    best[0] = 0
    for i in range(1, NWn + 1):
        for j in range(i - 1, -1, -1):
            gw = i - j
            if gw > 24:
                break
            L = int(max(Lw[j:i])); H = int(max(Hw[j:i]))
            if max(L, H) * gw * 128 > TOKCAP:
                break
            c = -(-(L * gw * 128) // MAXIDX) + -(-(H * gw * 128) // MAXIDX) + 3
            if best[j] + c < best[i]:
                best[i] = best[j] + c
                prev[i] = j
    bounds = []
    i = NWn
    while i > 0:
        bounds.append((prev[i], i))
        i = prev[i]
    bounds.reverse()
    groups = []
    for j, i in bounds:
        gw = i - j
        L = int(max(Lw[j:i])); H = int(max(Hw[j:i]))
        groups.append((j, gw, L, H))
    return groups


def _host_prep(x, edge_index, W1, b1, W2, b2):
    x = np.asarray(x, dtype=np.float32)
    ei = np.asarray(edge_index)
    W1 = np.asarray(W1, dtype=np.float32)
    W2 = np.asarray(W2, dtype=np.float32)
    b1 = np.asarray(b1, dtype=np.float32)
    b2 = np.asarray(b2, dtype=np.float32)

    loops = np.arange(N, dtype=np.int64)
    src = np.concatenate([ei[0].astype(np.int64), loops])
    dst = np.concatenate([ei[1].astype(np.int64), loops])

    deg = np.bincount(src, minlength=N).astype(np.float32)
    dis = np.power(deg, np.float32(-0.5), dtype=np.float32)
    dis[deg == 0] = 0.0

    # ---- node permutation ----
    indeg = np.bincount(dst, minlength=N)
    order_g = np.argsort(-indeg, kind="stable")
    core_of = np.empty(N, np.int64)
    core_of[order_g] = np.arange(N) % NCORES

    sstream = (core_of[src] >= LOCORES).astype(np.int64)
    dlo = np.bincount(dst[sstream == 0], minlength=N)
    dhi = np.bincount(dst[sstream == 1], minlength=N)

    n_of = np.empty(N, np.int64)   # position 0..6249 within core
    mul, muh = max(dlo.mean(), 1e-9), max(dhi.mean(), 1e-9)
    crit = np.maximum(dlo / mul, dhi / muh)
    core_nodes = []
    for c in range(NCORES):
        mine = np.where(core_of == c)[0]
        o = mine[np.argsort(-crit[mine], kind="stable")]
        n_of[o] = np.arange(NPC)
        core_nodes.append(o)

    row = core_of * NPAD + n_of    # permuted gather-table row per node

    # per-window global pads
    w_of, p_of = n_of // 128, n_of % 128
    Lw = np.zeros(NW, np.int64)
    Hw = np.zeros(NW, np.int64)
    np.maximum.at(Lw, w_of, dlo)
    np.maximum.at(Hw, w_of, dhi)
    groups = _plan_groups(Lw, Hw)

    # token offsets: per group, [lo block][hi block]
    g_off = []
    off = 0
    for (w0, gw, L, H) in groups:
        g_off.append((off, off + L * gw * 128))
        off += (L + H) * gw * 128
    TOKTOT = off
    assert TOKTOT % 16 == 0

    # group id / base window per window
    g_of_w = np.zeros(NW, np.int64)
    w0_of_w = np.zeros(NW, np.int64)
    for gi, (w0, gw, L, H) in enumerate(groups):
        g_of_w[w0:w0 + gw] = gi
        w0_of_w[w0:w0 + gw] = w0

    lo_off_arr = np.array([o[0] for o in g_off], np.int64)
    hi_off_arr = np.array([o[1] for o in g_off], np.int64)
    Lp_arr = np.array([g[2] for g in groups], np.int64)
    Hp_arr = np.array([g[3] for g in groups], np.int64)

    # chunk counts (for to_reg pooling)
    chunk_sets = set()
    for (w0, gw, L, H) in groups:
        for T in (L * gw * 128, H * gw * 128):
            nfull, rem = divmod(T, MAXIDX)
            if nfull:
                chunk_sets.add(MAXIDX)
            if rem:
                chunk_sets.add(rem)

    B1ZERO = bool(not b1.any())

    # base gidx filled with zero-row pointers
    gidx_base = np.empty(TOKTOT, np.int16)
    for gi, (w0, gw, L, H) in enumerate(groups):
        lo0, hi0 = g_off[gi]
        gidx_base[lo0:hi0] = ZLO
        gidx_base[hi0:hi0 + H * gw * 128] = ZHI

    in_maps = []
    for c in range(NCORES):
        sel = core_of[dst] == c
        s_c = src[sel]
        d_c = dst[sel]
        st_c = sstream[sel]
        n_c = n_of[d_c]
        key = n_c * 2 + st_c
        o2 = np.argsort(key, kind="stable")
        key_o = key[o2]
        first = np.searchsorted(key_o, key_o, side="left")
        rank = np.arange(len(key_o)) - first
        n_o = n_c[o2]
        st_o = st_c[o2]
        r_o = row[s_c[o2]]
        w_o = n_o // 128
        p_o = n_o % 128
        gi_o = g_of_w[w_o]
        b_o = w_o - w0_of_w[w_o]
        pad_o = np.where(st_o == 0, Lp_arr[gi_o], Hp_arr[gi_o])
        assert np.all(rank < pad_o)
        base_o = np.where(st_o == 0, lo_off_arr[gi_o], hi_off_arr[gi_o])
        pos = base_o + (b_o * pad_o + rank) * 128 + p_o
        val = np.where(st_o == 1, r_o - LOROWS, r_o).astype(np.int16)
        gidx = gidx_base.copy()
        gidx[pos] = val
        gidx_t = np.ascontiguousarray(np.tile(gidx.reshape(-1, 16).T, (8, 1)))

        nodes_c = np.full(NPAD, -1, np.int64)
        nodes_c[n_of[core_nodes[c]]] = core_nodes[c]
        valid = nodes_c >= 0
        nv = nodes_c[valid]

        dis_col = np.zeros(NPAD, np.float32)
        dis_col[valid] = dis[nv]
        dis2row = np.tile((dis_col * dis_col).astype(np.float16), (128, 1))
        disrow = np.tile(dis_col.astype(np.float16), (128, 1))

        xT16 = np.zeros((128, 2, NPAD), np.float16)
        xs = (x[nv] * dis_col[valid][:, None]).astype(np.float16)  # [6250, 256]
        xT16[:, 0, valid] = xs[:, 0:128].T
        xT16[:, 1, valid] = xs[:, 128:256].T

        im = {
            "gidx": gidx_t,
            "xT": np.ascontiguousarray(xT16.reshape(128, 2 * NPAD)),
            "W1": np.ascontiguousarray(
                W1.astype(np.float16).reshape(2, 128, FHID).transpose(1, 0, 2)
            ).reshape(128, 2 * FHID),
            "W2": W2.astype(np.float16),
            "b2v": b2.reshape(FOUT, 1).astype(np.float32),
            "dis2row": dis2row,
            "disrow": disrow,
        }
        if not B1ZERO:
            crow = np.zeros((128, NPAD), np.float32)
            dnz = dis_col[valid] > 0
            crow_cols = np.zeros(NPAD, np.float32)
            crow_cols[valid.nonzero()[0][dnz]] = 1.0 / dis_col[valid][dnz]
            crow = b1.reshape(FHID, 1) * crow_cols[None, :]
            im["crow"] = crow.astype(np.float32)
        in_maps.append(im)

    Kinfo = (tuple(groups), TOKTOT, B1ZERO, tuple(sorted(chunk_sets)),
             core_of, n_of)
    return in_maps, Kinfo


def _build(Kinfo):
    import concourse.bacc as bacc
    import concourse.mybir as mybir
    import concourse.tile as tile

    groups, TOKTOT, B1ZERO, chunk_counts = Kinfo[:4]
    PHASES = os.environ.get("GCN_PHASES", "full")
    REPEAT = int(os.environ.get("GCN_REPEAT", "1"))

    dt = mybir.dt
    ALU = mybir.AluOpType
    AXL = mybir.AxisListType

    nc = bacc.Bacc("TRN2", target_bir_lowering=False, debug=False,
                   num_devices=NCORES)

    gidx_d = nc.dram_tensor("gidx", [128, TOKTOT // 16], dt.int16, kind="ExternalInput")
    xT_d = nc.dram_tensor("xT", [128, 2 * NPAD], dt.float16, kind="ExternalInput")
    W1_d = nc.dram_tensor("W1", [128, 2 * FHID], dt.float16, kind="ExternalInput")
    W2_d = nc.dram_tensor("W2", [FHID, FOUT], dt.float16, kind="ExternalInput")
    b2v_d = nc.dram_tensor("b2v", [FOUT, 1], dt.float32, kind="ExternalInput")
    dis2row_d = nc.dram_tensor("dis2row", [128, NPAD], dt.float16, kind="ExternalInput")
    disrow_d = nc.dram_tensor("disrow", [128, NPAD], dt.float16, kind="ExternalInput")
    if not B1ZERO:
        crow_d = nc.dram_tensor("crow", [128, NPAD], dt.float32, kind="ExternalInput")
    out_d = nc.dram_tensor("out", [NPC, FOUT], dt.float32, kind="ExternalOutput")

    t1_local = nc.dram_tensor("t1_local", [NPAD, FHID], dt.float16)
    t1_full = nc.dram_tensor("t1_full", [NFULL, FHID], dt.float16, addr_space="Shared")
    t2_local = nc.dram_tensor("t2_local", [NPAD, FHID], dt.float16)
    t2_full = nc.dram_tensor("t2_full", [NFULL, FHID], dt.float16, addr_space="Shared")

    NMM = -(-NPAD // 512)  # 13 matmul groups of 512 nodes

    with tile.TileContext(nc) as tc:
        with (
            tc.tile_pool(name="consts", bufs=1) as cp,
            tc.tile_pool(name="psum", bufs=1, space="PSUM") as pp,
        ):
            w1_t = cp.tile([128, 2, FHID], dt.float16, tag="w1")
            nc.sync.dma_start(w1_t[:], W1_d[:, :].rearrange("p (k f) -> p k f", k=2))
            w2_t = cp.tile([FHID, FOUT], dt.float16, tag="w2")
            nc.sync.dma_start(w2_t[:], W2_d[:, :])
            b2v_t = cp.tile([FOUT, 1], dt.float32, tag="b2v")
            nc.sync.dma_start(b2v_t[:], b2v_d[:, :])
            dis2row_t = cp.tile([128, NPAD], dt.float16, tag="dis2row")
            nc.sync.dma_start(dis2row_t[:], dis2row_d[:, :])
            disrow_t = cp.tile([128, NPAD], dt.float16, tag="disrow")
            nc.sync.dma_start(disrow_t[:], disrow_d[:, :])
            gidx_t = cp.tile([128, TOKTOT // 16], dt.int16, tag="gidx")
            nc.sync.dma_start(gidx_t[:], gidx_d[:, :])
            if not B1ZERO:
                crow_t = cp.tile([128, NPAD], dt.float32, tag="crow")
                nc.sync.dma_start(crow_t[:], crow_d[:, :])

            nidx_regs = {cnt: nc.gpsimd.to_reg(cnt) for cnt in chunk_counts}

            def agg_layer(lname, src_full, red_all, redg, tokbuf):
                """Gather+reduce all groups of one layer into red_all."""
                base_lo = src_full[0:LOROWS, :]
                base_hi = src_full[LOROWS:NFULL, :]
                goff = 0
                for (w0, gw, L, H) in groups:
                    for s, (pad, base) in enumerate(((L, base_lo), (H, base_hi))):
                        T = pad * gw * 128
                        o = 0
                        while o < T:
                            cnt = min(MAXIDX, T - o)
                            c0 = (goff + o) // 16
                            nc.gpsimd.dma_gather(
                                tokbuf[:, 0:1, o:o + cnt], base,
                                gidx_t[:, c0:c0 + cnt // 16],
                                num_idxs=cnt, num_idxs_reg=nidx_regs[cnt],
                                elem_size=FHID, single_packet=False,
                                transpose=True)
                            o += cnt
                        red_out = (red_all if s == 0 else redg)
                        col0 = (w0 * 128 if s == 0 else 0)
                        nc.vector.tensor_reduce(
                            red_out[:, col0:col0 + gw * 128]
                            .rearrange("f (b p) -> f b p", b=gw),
                            tokbuf[:, 0, 0:T]
                            .rearrange("f (b k p) -> f b p k", b=gw, p=128),
                            AXL.X, ALU.add)
                        goff += T
                    nc.vector.tensor_tensor(
                        red_all[:, w0 * 128:(w0 + gw) * 128],
                        red_all[:, w0 * 128:(w0 + gw) * 128],
                        redg[:, 0:gw * 128], ALU.add)

            for _rep in range(REPEAT):
                # ---- phase B: h1T = W1^T @ (dis*x)^T, f-major ----
                with tc.tile_pool(name="phaseB", bufs=1) as pb:
                    xT_t = pb.tile([128, 2, NPAD], dt.float16, tag="xT")
                    nc.sync.dma_start(
                        xT_t[:], xT_d[:, :].rearrange("p (k n) -> p k n", k=2))
                    h1T = pb.tile([128, NPAD], dt.float16, tag="h1T")
                    psB = pp.tile([128, 4, 512], dt.float32, tag="pB")
                    for gi in range(NMM):
                        n0 = gi * 512
                        cols = min(512, NPAD - n0)
                        sl = psB[:, gi % 4, 0:cols]
                        for kc in range(2):
                            nc.tensor.matmul(
                                sl, w1_t[:, kc, :],
                                xT_t[:, kc, n0:n0 + cols],
                                start=(kc == 0), stop=(kc == 1))
                        if gi % 4 == 3:
                            nc.vector.tensor_copy(
                                h1T[:, (gi - 3) * 512:(gi + 1) * 512], psB[:])
                        elif gi == NMM - 1:
                            nc.vector.tensor_copy(
                                h1T[:, (gi // 4) * 4 * 512:NPAD],
                                psB[:, 0:(gi % 4) + 1, 0:cols])
                    stage = pb.tile([128, NW, FHID], dt.float16, tag="stageB")
                    nc.sync.dma_start(stage[:], h1T[:], transpose=True)
                    nc.sync.dma_start(
                        t1_local[:, :].rearrange("(s p) f -> p s f", p=128),
                        stage[:])

                nc.gpsimd.collective_compute(
                    "AllGather", mybir.AluOpType.bypass,
                    replica_groups=[list(range(NCORES))],
                    ins=[t1_local[:, :]], outs=[t1_full[:, :]],
                )

                if PHASES == "B":
                    with tc.tile_pool(name="dummy", bufs=1) as dp:
                        ot = dp.tile([128, NW * FOUT], dt.float32, tag="o")
                        nc.vector.memset(ot[:], 0.0)
                        nc.sync.dma_start(
                            out_d[0:48 * 128, :].rearrange("(s p) f -> p s f", p=128),
                            ot[:, 0:48 * FOUT].rearrange("p (s f) -> p s f", f=FOUT))
                        nc.sync.dma_start(out_d[48 * 128:NPC, :],
                                          ot[0:NPC - 48 * 128, 48 * FOUT:])
                    continue

                # ---- L1 aggregation ----
                with tc.tile_pool(name="L1", bufs=1) as l1:
                    tokbuf = l1.tile([128, 1, TOKCAP], dt.float16, tag="tok1")
                    red_all = l1.tile([128, NPAD], dt.float32, tag="red1")
                    redg = l1.tile([128, 24 * 128], dt.float32, tag="redg1")
                    o1T = l1.tile([128, NPAD], dt.float16, tag="o1T")
                    agg_layer("L1", t1_full, red_all, redg, tokbuf)
                    if not B1ZERO:
                        nc.vector.tensor_tensor(
                            red_all[:], red_all[:], crow_t[:], ALU.add)
                    nc.vector.scalar_tensor_tensor(
                        o1T[:], red_all[:], 0.0, dis2row_t[:],
                        ALU.max, ALU.mult)
                    stage1 = l1.tile([128, NW, FHID], dt.float16, tag="stage1")
                    nc.sync.dma_start(stage1[:], o1T[:], transpose=True)
                    nc.sync.dma_start(
                        t2_local[:, :].rearrange("(s p) f -> p s f", p=128),
                        stage1[:])

                if PHASES == "B1":
                    with tc.tile_pool(name="dummy2", bufs=1) as dp:
                        ot = dp.tile([128, NW * FOUT], dt.float32, tag="o")
                        nc.vector.memset(ot[:], 0.0)
                        nc.sync.dma_start(
                            out_d[0:48 * 128, :].rearrange("(s p) f -> p s f", p=128),
                            ot[:, 0:48 * FOUT].rearrange("p (s f) -> p s f", f=FOUT))
                        nc.sync.dma_start(out_d[48 * 128:NPC, :],
                                          ot[0:NPC - 48 * 128, 48 * FOUT:])
                    continue

                nc.gpsimd.collective_compute(
                    "AllGather", mybir.AluOpType.bypass,
                    replica_groups=[list(range(NCORES))],
                    ins=[t2_local[:, :]], outs=[t2_full[:, :]],
                )

                # ---- L2: aggregate o1, then @W2 + b2 ----
                with tc.tile_pool(name="L2", bufs=1) as l2:
                    tokbuf = l2.tile([128, 1, TOKCAP], dt.float16, tag="tok2")
                    red_all = l2.tile([128, NPAD], dt.float32, tag="red2")
                    redg = l2.tile([128, 24 * 128], dt.float32, tag="redg2")
                    r2T = l2.tile([128, NPAD], dt.float16, tag="r2T")
                    agg_layer("L2", t2_full, red_all, redg, tokbuf)
                    nc.vector.tensor_tensor(
                        r2T[:], red_all[:], disrow_t[:], ALU.mult)
                    h2T = l2.tile([128, NPAD], dt.float16, tag="h2T")
                    ps2 = pp.tile([128, 4, 512], dt.float32, tag="p2")
                    for gi in range(NMM):
                        n0 = gi * 512
                        cols = min(512, NPAD - n0)
                        nc.tensor.matmul(
                            ps2[0:FOUT, gi % 4, 0:cols], w2_t[:],
                            r2T[:, n0:n0 + cols], start=True, stop=True)
                        if gi % 4 == 3:
                            nc.vector.tensor_scalar(
                                h2T[0:FOUT, (gi - 3) * 512:(gi + 1) * 512],
                                ps2[0:FOUT, :, :], b2v_t[:, 0:1], None, ALU.add)
                        elif gi == NMM - 1:
                            nc.vector.tensor_scalar(
                                h2T[0:FOUT, (gi // 4) * 4 * 512:NPAD],
                                ps2[0:FOUT, 0:(gi % 4) + 1, 0:cols],
                                b2v_t[:, 0:1], None, ALU.add)
                    stage2 = l2.tile([128, NW, FOUT], dt.float16, tag="stage2")
                    nc.sync.dma_start(stage2[:], h2T[0:FOUT, :], transpose=True)
                    outst = l2.tile([128, NW, FOUT], dt.float32, tag="outst")
                    nc.vector.tensor_copy(outst[:], stage2[:])
                    nc.sync.dma_start(
                        out_d[0:48 * 128, :].rearrange("(s p) f -> p s f", p=128),
                        outst[:, 0:48, :])
                    nc.sync.dma_start(out_d[48 * 128:NPC, :],
                                      outst[0:NPC - 48 * 128, 48, :])

    nc.compile()
    return nc


def kernel(x, edge_index, W1, b1, W2, b2):
    global LAST_RESULTS
    from concourse.bass_utils import run_bass_kernel_spmd

    in_maps, Kinfo = _host_prep(x, edge_index, W1, b1, W2, b2)
    key = Kinfo[:4]
    if key not in _CACHE:
        _CACHE[key] = _build(Kinfo)
    nc = _CACHE[key]

    res = run_bass_kernel_spmd(nc, in_maps, list(range(NCORES)))
    LAST_RESULTS = res

    core_of, n_of = Kinfo[4], Kinfo[5]
    out = np.empty((N, FOUT), np.float32)
    for c in range(NCORES):
        mine = np.where(core_of == c)[0]
        out[mine] = res.results[c]["out"][n_of[mine]]
    return out


# revision 20
# speedup vs baseline: 3.0111x; 2.0741x over previous
"""GCN encoder (2-layer) on 8 Trainium2 NeuronCores — instruction-minimal design.

This environment executes roughly one engine instruction per ~55-67us with no
cross-engine overlap, so the design minimizes instruction count:

  - f-major compute: h1T = W1^T @ xT with nodes as the matmul free dim
    (512 nodes/matmul -> 26 matmuls vs 98 node-major), DMA-transpose (xbar)
    converts f-major SBUF tiles to node-major DRAM gather tables (fp16).
  - transpose-mode dma_gather (fp16, elem=128) yields tokens in [feat, token]
    layout; one strided 4D-AP tensor_reduce aggregates a whole multi-window
    group; whole-layer scalar_tensor_tensor applies relu/deg scaling.
  - host-side node permutation: nodes are dealt to cores by sorted in-degree
    and slotted within a core to balance per-(group,stream) max rank, cutting
    gather padding tokens ~33%; host un-permutes the final output for free.
  - lo/hi gather-base split at a core boundary (5/3) keeps int16 indices
    valid while making each edge's stream invariant to within-core slotting.
  - group boundaries chosen by DP minimizing gathers + reduce overhead under
    the SBUF token-buffer cap.

Sharding: nodes dealt 6250/core (permuted), edges partitioned by dst core,
weights replicated, fp16 AllGather between layers.
"""
import os
import numpy as np

N, E = 50000, 1600000
FIN, FHID, FOUT = 256, 128, 64
NCORES = 8
NPC = N // NCORES          # 6250
NW = 49                    # windows per core
NPAD = NW * 128            # 6272
NFULL = NCORES * NPAD      # 50176
LOCORES = 5
LOROWS = LOCORES * NPAD    # 31360 rows in the lo gather base (< 32768)
ZROW = 106 * NW + 48       # all-zero pad row (node 6250), core-local p-major
ZLO = ZROW                 # zero row inside lo base (core 0)
ZHI = ZROW                 # core 5 zero row, hi-base-local
MAXIDX = 8192              # max indices per dma_gather instruction
TOKCAP = 30720             # token-buffer capacity (60KB fp16 per partition)

_CACHE = {}
LAST_RESULTS = None


def _plan_groups(Lw, Hw):
    """DP over sorted windows: pick group boundaries minimizing
    gathers + 3 per group (2 reduces + 1 add), under TOKCAP."""
    NWn = len(Lw)
    INF = 1 << 30
    best = [INF] * (NWn + 1)
    prev = [0] * (NWn + 1)
    best[0] = 0
    for i in range(1, NWn + 1):
        for j in range(i - 1, -1, -1):
            gw = i - j
            if gw > 24:
                break
            L = int(max(Lw[j:i])); H = int(max(Hw[j:i]))
            if max(L, H) * gw * 128 > TOKCAP:
                break
            c = -(-(L * gw * 128) // MAXIDX) + -(-(H * gw * 128) // MAXIDX) + 3
            if best[j] + c < best[i]:
                best[i] = best[j] + c
                prev[i] = j
    bounds = []
    i = NWn
    while i > 0:
        bounds.append((prev[i], i))
        i = prev[i]
    bounds.reverse()
    groups = []
    for j, i in bounds:
        gw = i - j
        L = int(max(Lw[j:i])); H = int(max(Hw[j:i]))
        groups.append((j, gw, L, H))
    return groups


def _host_prep(x, edge_index, W1, b1, W2, b2):
    x = np.asarray(x, dtype=np.float32)
    ei = np.asarray(edge_index)
    W1 = np.asarray(W1, dtype=np.float32)
    W2 = np.asarray(W2, dtype=np.float32)
    b1 = np.asarray(b1, dtype=np.float32)
    b2 = np.asarray(b2, dtype=np.float32)

    loops = np.arange(N, dtype=np.int64)
    src = np.concatenate([ei[0].astype(np.int64), loops])
    dst = np.concatenate([ei[1].astype(np.int64), loops])

    deg = np.bincount(src, minlength=N).astype(np.float32)
    dis = np.power(deg, np.float32(-0.5), dtype=np.float32)
    dis[deg == 0] = 0.0

    # ---- node permutation ----
    indeg = np.bincount(dst, minlength=N)
    order_g = np.argsort(-indeg, kind="stable")
    core_of = np.empty(N, np.int64)
    core_of[order_g] = np.arange(N) % NCORES

    sstream = (core_of[src] >= LOCORES).astype(np.int64)
    dlo = np.bincount(dst[sstream == 0], minlength=N)
    dhi = np.bincount(dst[sstream == 1], minlength=N)

    n_of = np.empty(N, np.int64)   # position 0..6249 within core
    mul, muh = max(dlo.mean(), 1e-9), max(dhi.mean(), 1e-9)
    crit = np.maximum(dlo / mul, dhi / muh)
    core_nodes = []
    for c in range(NCORES):
        mine = np.where(core_of == c)[0]
        o = mine[np.argsort(-crit[mine], kind="stable")]
        n_of[o] = np.arange(NPC)
        core_nodes.append(o)

    # p-major table rows: node at (c, n) with n = w*128+p sits at DRAM row
    # c*NPAD + p*NW + w, so the dma-transpose stage [p, w, f] writes the
    # table contiguously (no scatter descriptors).
    row = core_of * NPAD + (n_of % 128) * NW + n_of // 128

    # per-window global pads
    w_of, p_of = n_of // 128, n_of % 128
    Lw = np.zeros(NW, np.int64)
    Hw = np.zeros(NW, np.int64)
    np.maximum.at(Lw, w_of, dlo)
    np.maximum.at(Hw, w_of, dhi)
    groups = _plan_groups(Lw, Hw)

    # token offsets: per group, [lo block][hi block]
    g_off = []
    off = 0
    for (w0, gw, L, H) in groups:
        g_off.append((off, off + L * gw * 128))
        off += (L + H) * gw * 128
    TOKTOT = off
    assert TOKTOT % 16 == 0

    # group id / base window per window
    g_of_w = np.zeros(NW, np.int64)
    w0_of_w = np.zeros(NW, np.int64)
    for gi, (w0, gw, L, H) in enumerate(groups):
        g_of_w[w0:w0 + gw] = gi
        w0_of_w[w0:w0 + gw] = w0

    lo_off_arr = np.array([o[0] for o in g_off], np.int64)
    hi_off_arr = np.array([o[1] for o in g_off], np.int64)
    Lp_arr = np.array([g[2] for g in groups], np.int64)
    Hp_arr = np.array([g[3] for g in groups], np.int64)

    # chunk counts (for to_reg pooling)
    chunk_sets = set()
    for (w0, gw, L, H) in groups:
        for T in (L * gw * 128, H * gw * 128):
            nfull, rem = divmod(T, MAXIDX)
            if nfull:
                chunk_sets.add(MAXIDX)
            if rem:
                chunk_sets.add(rem)

    B1ZERO = bool(not b1.any())

    # base gidx filled with zero-row pointers
    gidx_base = np.empty(TOKTOT, np.int16)
    for gi, (w0, gw, L, H) in enumerate(groups):
        lo0, hi0 = g_off[gi]
        gidx_base[lo0:hi0] = ZLO
        gidx_base[hi0:hi0 + H * gw * 128] = ZHI

    in_maps = []
    for c in range(NCORES):
        sel = core_of[dst] == c
        s_c = src[sel]
        d_c = dst[sel]
        st_c = sstream[sel]
        n_c = n_of[d_c]
        key = n_c * 2 + st_c
        o2 = np.argsort(key, kind="stable")
        key_o = key[o2]
        first = np.searchsorted(key_o, key_o, side="left")
        rank = np.arange(len(key_o)) - first
        n_o = n_c[o2]
        st_o = st_c[o2]
        r_o = row[s_c[o2]]
        w_o = n_o // 128
        p_o = n_o % 128
        gi_o = g_of_w[w_o]
        b_o = w_o - w0_of_w[w_o]
        pad_o = np.where(st_o == 0, Lp_arr[gi_o], Hp_arr[gi_o])
        assert np.all(rank < pad_o)
        base_o = np.where(st_o == 0, lo_off_arr[gi_o], hi_off_arr[gi_o])
        pos = base_o + (b_o * pad_o + rank) * 128 + p_o
        val = np.where(st_o == 1, r_o - LOROWS, r_o).astype(np.int16)
        gidx = gidx_base.copy()
        gidx[pos] = val
        gidx_t = np.ascontiguousarray(np.tile(gidx.reshape(-1, 16).T, (8, 1)))

        nodes_c = np.full(NPAD, -1, np.int64)
        nodes_c[n_of[core_nodes[c]]] = core_nodes[c]
        valid = nodes_c >= 0
        nv = nodes_c[valid]

        dis_col = np.zeros(NPAD, np.float32)
        dis_col[valid] = dis[nv]
        dis2row = np.tile((dis_col * dis_col).astype(np.float16), (128, 1))
        disrow = np.tile(dis_col.astype(np.float16), (128, 1))

        xT16 = np.zeros((128, 2, NPAD), np.float16)
        xs = (x[nv] * dis_col[valid][:, None]).astype(np.float16)  # [6250, 256]
        xT16[:, 0, valid] = xs[:, 0:128].T
        xT16[:, 1, valid] = xs[:, 128:256].T

        im = {
            "gidx": gidx_t,
            "xT": np.ascontiguousarray(xT16.reshape(128, 2 * NPAD)),
            "W1": np.ascontiguousarray(
                W1.astype(np.float16).reshape(2, 128, FHID).transpose(1, 0, 2)
            ).reshape(128, 2 * FHID),
            "W2": W2.astype(np.float16),
            "b2v": b2.reshape(FOUT, 1).astype(np.float32),
            "dis2row": dis2row,
            "disrow": disrow,
        }
        if not B1ZERO:
            crow = np.zeros((128, NPAD), np.float32)
            dnz = dis_col[valid] > 0
            crow_cols = np.zeros(NPAD, np.float32)
            crow_cols[valid.nonzero()[0][dnz]] = 1.0 / dis_col[valid][dnz]
            crow = b1.reshape(FHID, 1) * crow_cols[None, :]
            im["crow"] = crow.astype(np.float32)
        in_maps.append(im)

    Kinfo = (tuple(groups), TOKTOT, B1ZERO, tuple(sorted(chunk_sets)),
             core_of, n_of)
    return in_maps, Kinfo


def _build(Kinfo):
    import concourse.bacc as bacc
    import concourse.mybir as mybir
    import concourse.tile as tile

    groups, TOKTOT, B1ZERO, chunk_counts = Kinfo[:4]
    PHASES = os.environ.get("GCN_PHASES", "full")
    REPEAT = int(os.environ.get("GCN_REPEAT", "1"))
    SKIP_GATHER = bool(int(os.environ.get("GCN_SKIP_GATHER", "0")))
    SKIP_AG = bool(int(os.environ.get("GCN_SKIP_AG", "0")))

    dt = mybir.dt
    ALU = mybir.AluOpType
    AXL = mybir.AxisListType

    nc = bacc.Bacc("TRN2", target_bir_lowering=False, debug=False,
                   num_devices=NCORES)

    gidx_d = nc.dram_tensor("gidx", [128, TOKTOT // 16], dt.int16, kind="ExternalInput")
    xT_d = nc.dram_tensor("xT", [128, 2 * NPAD], dt.float16, kind="ExternalInput")
    W1_d = nc.dram_tensor("W1", [128, 2 * FHID], dt.float16, kind="ExternalInput")
    W2_d = nc.dram_tensor("W2", [FHID, FOUT], dt.float16, kind="ExternalInput")
    b2v_d = nc.dram_tensor("b2v", [FOUT, 1], dt.float32, kind="ExternalInput")
    dis2row_d = nc.dram_tensor("dis2row", [128, NPAD], dt.float16, kind="ExternalInput")
    disrow_d = nc.dram_tensor("disrow", [128, NPAD], dt.float16, kind="ExternalInput")
    if not B1ZERO:
        crow_d = nc.dram_tensor("crow", [128, NPAD], dt.float32, kind="ExternalInput")
    out_d = nc.dram_tensor("out", [128, NW * FOUT], dt.float32, kind="ExternalOutput")

    t1_local = nc.dram_tensor("t1_local", [NPAD, FHID], dt.float16)
    t1_full = nc.dram_tensor("t1_full", [NFULL, FHID], dt.float16, addr_space="Shared")
    t2_local = nc.dram_tensor("t2_local", [NPAD, FHID], dt.float16)
    t2_full = nc.dram_tensor("t2_full", [NFULL, FHID], dt.float16, addr_space="Shared")

    NMM = -(-NPAD // 512)  # 13 matmul groups of 512 nodes

    with tile.TileContext(nc) as tc:
        with (
            tc.tile_pool(name="consts", bufs=1) as cp,
            tc.tile_pool(name="psum", bufs=1, space="PSUM") as pp,
        ):
            w1_t = cp.tile([128, 2, FHID], dt.float16, tag="w1")
            nc.sync.dma_start(w1_t[:], W1_d[:, :].rearrange("p (k f) -> p k f", k=2))
            w2_t = cp.tile([FHID, FOUT], dt.float16, tag="w2")
            nc.sync.dma_start(w2_t[:], W2_d[:, :])
            b2v_t = cp.tile([FOUT, 1], dt.float32, tag="b2v")
            nc.sync.dma_start(b2v_t[:], b2v_d[:, :])
            dis2row_t = cp.tile([128, NPAD], dt.float16, tag="dis2row")
            nc.sync.dma_start(dis2row_t[:], dis2row_d[:, :])
            disrow_t = cp.tile([128, NPAD], dt.float16, tag="disrow")
            nc.sync.dma_start(disrow_t[:], disrow_d[:, :])
            gidx_t = cp.tile([128, TOKTOT // 16], dt.int16, tag="gidx")
            nc.sync.dma_start(gidx_t[:], gidx_d[:, :])
            if not B1ZERO:
                crow_t = cp.tile([128, NPAD], dt.float32, tag="crow")
                nc.sync.dma_start(crow_t[:], crow_d[:, :])

            nidx_regs = {cnt: nc.gpsimd.to_reg(cnt) for cnt in chunk_counts}

            def agg_layer(lname, src_full, red_all, redg, tokbuf):
                """Gather+reduce all groups of one layer into red_all."""
                base_lo = src_full[0:LOROWS, :]
                base_hi = src_full[LOROWS:NFULL, :]
                goff = 0
                for (w0, gw, L, H) in groups:
                    for s, (pad, base) in enumerate(((L, base_lo), (H, base_hi))):
                        T = pad * gw * 128
                        o = 0
                        while o < T and not SKIP_GATHER:
                            cnt = min(MAXIDX, T - o)
                            c0 = (goff + o) // 16
                            nc.gpsimd.dma_gather(
                                tokbuf[:, 0:1, o:o + cnt], base,
                                gidx_t[:, c0:c0 + cnt // 16],
                                num_idxs=cnt, num_idxs_reg=nidx_regs[cnt],
                                elem_size=FHID, single_packet=False,
                                transpose=True)
                            o += cnt
                        red_out = (red_all if s == 0 else redg)
                        col0 = (w0 * 128 if s == 0 else 0)
                        nc.vector.tensor_reduce(
                            red_out[:, col0:col0 + gw * 128]
                            .rearrange("f (b p) -> f b p", b=gw),
                            tokbuf[:, 0, 0:T]
                            .rearrange("f (b k p) -> f b p k", b=gw, p=128),
                            AXL.X, ALU.add)
                        goff += T
                    nc.vector.tensor_tensor(
                        red_all[:, w0 * 128:(w0 + gw) * 128],
                        red_all[:, w0 * 128:(w0 + gw) * 128],
                        redg[:, 0:gw * 128], ALU.add)

            for _rep in range(REPEAT):
                # ---- phase B: h1T = W1^T @ (dis*x)^T, f-major ----
                with tc.tile_pool(name="phaseB", bufs=1) as pb:
                    xT_t = pb.tile([128, 2, NPAD], dt.float16, tag="xT")
                    nc.sync.dma_start(
                        xT_t[:], xT_d[:, :].rearrange("p (k n) -> p k n", k=2))
                    h1T = pb.tile([128, NPAD], dt.float16, tag="h1T")
                    psB = pp.tile([128, 4, 512], dt.float32, tag="pB")
                    for gi in range(NMM):
                        n0 = gi * 512
                        cols = min(512, NPAD - n0)
                        sl = psB[:, gi % 4, 0:cols]
                        for kc in range(2):
                            nc.tensor.matmul(
                                sl, w1_t[:, kc, :],
                                xT_t[:, kc, n0:n0 + cols],
                                start=(kc == 0), stop=(kc == 1))
                        if gi % 4 == 3:
                            nc.vector.tensor_copy(
                                h1T[:, (gi - 3) * 512:(gi + 1) * 512], psB[:])
                        elif gi == NMM - 1:
                            nc.vector.tensor_copy(
                                h1T[:, (gi // 4) * 4 * 512:NPAD],
                                psB[:, 0:(gi % 4) + 1, 0:cols])
                    stage = pb.tile([128, NW, FHID], dt.float16, tag="stageB")
                    nc.sync.dma_start(stage[:], h1T[:], transpose=True)
                    nc.sync.dma_start(
                        t1_local[:, :].rearrange("(p s) f -> p s f", p=128),
                        stage[:])

                if not SKIP_AG:
                    nc.gpsimd.collective_compute(
                        "AllGather", mybir.AluOpType.bypass,
                        replica_groups=[list(range(NCORES))],
                        ins=[t1_local[:, :]], outs=[t1_full[:, :]],
                    )

                if PHASES in ("B", "B0"):
                    with tc.tile_pool(name="dummy", bufs=1) as dp:
                        ot = dp.tile([128, NW * FOUT], dt.float32, tag="o")
                        nc.vector.memset(ot[:], 0.0)
                        nc.sync.dma_start(out_d[:, :], ot[:])
                    continue

                # ---- L1 aggregation ----
                with tc.tile_pool(name="L1", bufs=1) as l1:
                    tokbuf = l1.tile([128, 1, TOKCAP], dt.float16, tag="tok1")
                    red_all = l1.tile([128, NPAD], dt.float32, tag="red1")
                    redg = l1.tile([128, 24 * 128], dt.float32, tag="redg1")
                    o1T = l1.tile([128, NPAD], dt.float16, tag="o1T")
                    agg_layer("L1", t1_full, red_all, redg, tokbuf)
                    if not B1ZERO:
                        nc.vector.tensor_tensor(
                            red_all[:], red_all[:], crow_t[:], ALU.add)
                    nc.vector.scalar_tensor_tensor(
                        o1T[:], red_all[:], 0.0, dis2row_t[:],
                        ALU.max, ALU.mult)
                    stage1 = l1.tile([128, NW, FHID], dt.float16, tag="stage1")
                    nc.sync.dma_start(stage1[:], o1T[:], transpose=True)
                    nc.sync.dma_start(
                        t2_local[:, :].rearrange("(p s) f -> p s f", p=128),
                        stage1[:])

                if PHASES == "B1":
                    with tc.tile_pool(name="dummy2", bufs=1) as dp:
                        ot = dp.tile([128, NW * FOUT], dt.float32, tag="o")
                        nc.vector.memset(ot[:], 0.0)
                        nc.sync.dma_start(out_d[:, :], ot[:])
                    continue

                if not SKIP_AG:
                    nc.gpsimd.collective_compute(
                        "AllGather", mybir.AluOpType.bypass,
                        replica_groups=[list(range(NCORES))],
                        ins=[t2_local[:, :]], outs=[t2_full[:, :]],
                    )

                # ---- L2: aggregate o1, then @W2 + b2 ----
                with tc.tile_pool(name="L2", bufs=1) as l2:
                    tokbuf = l2.tile([128, 1, TOKCAP], dt.float16, tag="tok2")
                    red_all = l2.tile([128, NPAD], dt.float32, tag="red2")
                    redg = l2.tile([128, 24 * 128], dt.float32, tag="redg2")
                    r2T = l2.tile([128, NPAD], dt.float16, tag="r2T")
                    agg_layer("L2", t2_full, red_all, redg, tokbuf)
                    nc.vector.tensor_tensor(
                        r2T[:], red_all[:], disrow_t[:], ALU.mult)
                    h2T = l2.tile([128, NPAD], dt.float16, tag="h2T")
                    ps2 = pp.tile([128, 4, 512], dt.float32, tag="p2")
                    for gi in range(NMM):
                        n0 = gi * 512
                        cols = min(512, NPAD - n0)
                        nc.tensor.matmul(
                            ps2[0:FOUT, gi % 4, 0:cols], w2_t[:],
                            r2T[:, n0:n0 + cols], start=True, stop=True)
                        if gi % 4 == 3:
                            nc.vector.tensor_scalar(
                                h2T[0:FOUT, (gi - 3) * 512:(gi + 1) * 512],
                                ps2[0:FOUT, :, :], b2v_t[:, 0:1], None, ALU.add)
                        elif gi == NMM - 1:
                            nc.vector.tensor_scalar(
                                h2T[0:FOUT, (gi // 4) * 4 * 512:NPAD],
                                ps2[0:FOUT, 0:(gi % 4) + 1, 0:cols],
                                b2v_t[:, 0:1], None, ALU.add)
                    stage2 = l2.tile([128, NW, FOUT], dt.float16, tag="stage2")
                    nc.sync.dma_start(stage2[:], h2T[0:FOUT, :], transpose=True)
                    outst = l2.tile([128, NW, FOUT], dt.float32, tag="outst")
                    nc.vector.tensor_copy(outst[:], stage2[:])
                    nc.sync.dma_start(out_d[:, :],
                                      outst[:].rearrange("p s f -> p (s f)"))

    nc.compile()
    return nc


def kernel(x, edge_index, W1, b1, W2, b2):
    global LAST_RESULTS
    from concourse.bass_utils import run_bass_kernel_spmd

    in_maps, Kinfo = _host_prep(x, edge_index, W1, b1, W2, b2)
    key = Kinfo[:4]
    if key not in _CACHE:
        _CACHE[key] = _build(Kinfo)
    nc = _CACHE[key]

    res = run_bass_kernel_spmd(nc, in_maps, list(range(NCORES)))
    LAST_RESULTS = res

    core_of, n_of = Kinfo[4], Kinfo[5]
    out = np.empty((N, FOUT), np.float32)
    for c in range(NCORES):
        mine = np.where(core_of == c)[0]
        arr = res.results[c]["out"].reshape(128, NW, FOUT)
        out[mine] = arr[n_of[mine] % 128, n_of[mine] // 128]
    return out


# revision 35
# speedup vs baseline: 3.9016x; 1.2958x over previous
"""GCN encoder (2-layer) on 8 Trainium2 NeuronCores — instruction-minimal design.

This environment executes roughly one engine instruction per ~55-67us with no
cross-engine overlap, so the design minimizes instruction count:

  - f-major compute: h1T = W1^T @ xT with nodes as the matmul free dim
    (512 nodes/matmul -> 26 matmuls vs 98 node-major), DMA-transpose (xbar)
    converts f-major SBUF tiles to node-major DRAM gather tables (fp16).
  - transpose-mode dma_gather (fp16, elem=128) yields tokens in [feat, token]
    layout; one strided 4D-AP tensor_reduce aggregates a whole multi-window
    group; whole-layer scalar_tensor_tensor applies relu/deg scaling.
  - host-side node permutation: nodes are dealt to cores by sorted in-degree
    and slotted within a core to balance per-(group,stream) max rank, cutting
    gather padding tokens ~33%; host un-permutes the final output for free.
  - lo/hi gather-base split at a core boundary (5/3) keeps int16 indices
    valid while making each edge's stream invariant to within-core slotting.
  - group boundaries chosen by DP minimizing gathers + reduce overhead under
    the SBUF token-buffer cap.

Sharding: nodes dealt 6250/core (permuted), edges partitioned by dst core,
weights replicated, fp16 AllGather between layers.
"""
import os
import numpy as np

N, E = 50000, 1600000
FIN, FHID, FOUT = 256, 128, 64
NCORES = 8
NPC = N // NCORES          # 6250
NW = 49                    # windows per core
NPAD = NW * 128            # 6272
NFULL = NCORES * NPAD      # 50176
LOCORES = 5
LOROWS = LOCORES * NPAD    # 31360 rows in the lo gather base (< 32768)
ZROW = 106 * NW + 48       # all-zero pad row (node 6250), core-local p-major
ZLO = ZROW                 # zero row inside lo base (core 0)
ZHI = ZROW                 # core 5 zero row, hi-base-local
MAXIDX = 8192              # max indices per dma_gather instruction
DBLBUF = bool(int(os.environ.get("GCN_DBLBUF", "0")))
TOKCAP = 18432 if DBLBUF else 30720  # token-buffer cap (fp16/partition)
GWCAP = 12 if DBLBUF else 24         # max windows per group (redg width)

_CACHE = {}
LAST_RESULTS = None


def _plan_groups(Lw, Hw):
    """DP over sorted windows: pick group boundaries minimizing
    gathers + 3 per group (2 reduces + 1 add), under TOKCAP."""
    NWn = len(Lw)
    INF = 1 << 30
    best = [INF] * (NWn + 1)
    prev = [0] * (NWn + 1)
    best[0] = 0
    for i in range(1, NWn + 1):
        for j in range(i - 1, -1, -1):
            gw = i - j
            if gw > GWCAP:
                break
            L = int(max(Lw[j:i])); H = int(max(Hw[j:i]))
            if max(L, H) * gw * 128 > TOKCAP:
                break
            c = -(-(L * gw * 128) // MAXIDX) + -(-(H * gw * 128) // MAXIDX) + 3
            if best[j] + c < best[i]:
                best[i] = best[j] + c
                prev[i] = j
    bounds = []
    i = NWn
    while i > 0:
        bounds.append((prev[i], i))
        i = prev[i]
    bounds.reverse()
    groups = []
    for j, i in bounds:
        gw = i - j
        L = int(max(Lw[j:i])); H = int(max(Hw[j:i]))
        groups.append((j, gw, L, H))
    return groups


def _host_prep(x, edge_index, W1, b1, W2, b2):
    x = np.asarray(x, dtype=np.float32)
    ei = np.asarray(edge_index)
    W1 = np.asarray(W1, dtype=np.float32)
    W2 = np.asarray(W2, dtype=np.float32)
    b1 = np.asarray(b1, dtype=np.float32)
    b2 = np.asarray(b2, dtype=np.float32)

    loops = np.arange(N, dtype=np.int64)
    src = np.concatenate([ei[0].astype(np.int64), loops])
    dst = np.concatenate([ei[1].astype(np.int64), loops])

    deg = np.bincount(src, minlength=N).astype(np.float32)
    dis = np.power(deg, np.float32(-0.5), dtype=np.float32)
    dis[deg == 0] = 0.0

    # ---- node permutation ----
    indeg = np.bincount(dst, minlength=N)
    order_g = np.argsort(-indeg, kind="stable")
    core_of = np.empty(N, np.int64)
    core_of[order_g] = np.arange(N) % NCORES

    sstream = (core_of[src] >= LOCORES).astype(np.int64)
    dlo = np.bincount(dst[sstream == 0], minlength=N)
    dhi = np.bincount(dst[sstream == 1], minlength=N)

    n_of = np.empty(N, np.int64)   # position 0..6249 within core
    mul, muh = max(dlo.mean(), 1e-9), max(dhi.mean(), 1e-9)
    crit = np.maximum(dlo / mul, dhi / muh)
    core_nodes = []
    for c in range(NCORES):
        mine = np.where(core_of == c)[0]
        o = mine[np.argsort(-crit[mine], kind="stable")]
        n_of[o] = np.arange(NPC)
        core_nodes.append(o)

    # p-major table rows: node at (c, n) with n = w*128+p sits at DRAM row
    # c*NPAD + p*NW + w, so the dma-transpose stage [p, w, f] writes the
    # table contiguously (no scatter descriptors).
    row = core_of * NPAD + (n_of % 128) * NW + n_of // 128

    # per-window global pads
    w_of, p_of = n_of // 128, n_of % 128
    Lw = np.zeros(NW, np.int64)
    Hw = np.zeros(NW, np.int64)
    np.maximum.at(Lw, w_of, dlo)
    np.maximum.at(Hw, w_of, dhi)
    groups = _plan_groups(Lw, Hw)

    # token offsets: per group, [lo block][hi block]
    g_off = []
    off = 0
    for (w0, gw, L, H) in groups:
        g_off.append((off, off + L * gw * 128))
        off += (L + H) * gw * 128
    TOKTOT = off
    assert TOKTOT % 16 == 0

    # group id / base window per window
    g_of_w = np.zeros(NW, np.int64)
    w0_of_w = np.zeros(NW, np.int64)
    for gi, (w0, gw, L, H) in enumerate(groups):
        g_of_w[w0:w0 + gw] = gi
        w0_of_w[w0:w0 + gw] = w0

    lo_off_arr = np.array([o[0] for o in g_off], np.int64)
    hi_off_arr = np.array([o[1] for o in g_off], np.int64)
    Lp_arr = np.array([g[2] for g in groups], np.int64)
    Hp_arr = np.array([g[3] for g in groups], np.int64)

    # chunk counts (for to_reg pooling)
    chunk_sets = set()
    for (w0, gw, L, H) in groups:
        for T in (L * gw * 128, H * gw * 128):
            nfull, rem = divmod(T, MAXIDX)
            if nfull:
                chunk_sets.add(MAXIDX)
            if rem:
                chunk_sets.add(rem)

    B1ZERO = bool(not b1.any())

    # base gidx filled with zero-row pointers
    gidx_base = np.empty(TOKTOT, np.int16)
    for gi, (w0, gw, L, H) in enumerate(groups):
        lo0, hi0 = g_off[gi]
        gidx_base[lo0:hi0] = ZLO
        gidx_base[hi0:hi0 + H * gw * 128] = ZHI

    in_maps = []
    for c in range(NCORES):
        sel = core_of[dst] == c
        s_c = src[sel]
        d_c = dst[sel]
        st_c = sstream[sel]
        n_c = n_of[d_c]
        key = n_c * 2 + st_c
        o2 = np.argsort(key, kind="stable")
        key_o = key[o2]
        first = np.searchsorted(key_o, key_o, side="left")
        rank = np.arange(len(key_o)) - first
        n_o = n_c[o2]
        st_o = st_c[o2]
        r_o = row[s_c[o2]]
        w_o = n_o // 128
        p_o = n_o % 128
        gi_o = g_of_w[w_o]
        b_o = w_o - w0_of_w[w_o]
        pad_o = np.where(st_o == 0, Lp_arr[gi_o], Hp_arr[gi_o])
        assert np.all(rank < pad_o)
        base_o = np.where(st_o == 0, lo_off_arr[gi_o], hi_off_arr[gi_o])
        pos = base_o + (b_o * pad_o + rank) * 128 + p_o
        val = np.where(st_o == 1, r_o - LOROWS, r_o).astype(np.int16)
        gidx = gidx_base.copy()
        gidx[pos] = val
        gidx_t = np.ascontiguousarray(np.tile(gidx.reshape(-1, 16).T, (8, 1)))

        nodes_c = np.full(NPAD, -1, np.int64)
        nodes_c[n_of[core_nodes[c]]] = core_nodes[c]
        valid = nodes_c >= 0
        nv = nodes_c[valid]

        dis_col = np.zeros(NPAD, np.float32)
        dis_col[valid] = dis[nv]
        dis2row = np.tile((dis_col * dis_col).astype(np.float16), (128, 1))
        disrow = np.tile(dis_col.astype(np.float16), (128, 1))

        xT16 = np.zeros((128, 2, NPAD), np.float16)
        xs = (x[nv] * dis_col[valid][:, None]).astype(np.float16)  # [6250, 256]
        xT16[:, 0, valid] = xs[:, 0:128].T
        xT16[:, 1, valid] = xs[:, 128:256].T

        im = {
            "gidx": gidx_t,
            "xT": np.ascontiguousarray(xT16.reshape(128, 2 * NPAD)),
            "W1": np.ascontiguousarray(
                W1.astype(np.float16).reshape(2, 128, FHID).transpose(1, 0, 2)
            ).reshape(128, 2 * FHID),
            "W2": W2.astype(np.float16),
            "b2v": b2.reshape(FOUT, 1).astype(np.float32),
            "dis2row": dis2row,
            "disrow": disrow,
        }
        if not B1ZERO:
            crow = np.zeros((128, NPAD), np.float32)
            dnz = dis_col[valid] > 0
            crow_cols = np.zeros(NPAD, np.float32)
            crow_cols[valid.nonzero()[0][dnz]] = 1.0 / dis_col[valid][dnz]
            crow = b1.reshape(FHID, 1) * crow_cols[None, :]
            im["crow"] = crow.astype(np.float32)
        in_maps.append(im)

    Kinfo = (tuple(groups), TOKTOT, B1ZERO, tuple(sorted(chunk_sets)),
             core_of, n_of)
    return in_maps, Kinfo


def _build(Kinfo):
    import concourse.bacc as bacc
    import concourse.mybir as mybir
    import concourse.tile as tile

    groups, TOKTOT, B1ZERO, chunk_counts = Kinfo[:4]
    PHASES = os.environ.get("GCN_PHASES", "full")
    REPEAT = int(os.environ.get("GCN_REPEAT", "1"))
    SKIP_GATHER = bool(int(os.environ.get("GCN_SKIP_GATHER", "0")))
    SKIP_AG = bool(int(os.environ.get("GCN_SKIP_AG", "0")))
    NQUEUES = int(os.environ.get("GCN_QUEUES", "1"))

    dt = mybir.dt
    ALU = mybir.AluOpType
    AXL = mybir.AxisListType

    nc = bacc.Bacc("TRN2", target_bir_lowering=False, debug=False,
                   num_devices=NCORES, num_swdge_queues=NQUEUES)

    gidx_d = nc.dram_tensor("gidx", [128, TOKTOT // 16], dt.int16, kind="ExternalInput")
    xT_d = nc.dram_tensor("xT", [128, 2 * NPAD], dt.float16, kind="ExternalInput")
    W1_d = nc.dram_tensor("W1", [128, 2 * FHID], dt.float16, kind="ExternalInput")
    W2_d = nc.dram_tensor("W2", [FHID, FOUT], dt.float16, kind="ExternalInput")
    b2v_d = nc.dram_tensor("b2v", [FOUT, 1], dt.float32, kind="ExternalInput")
    dis2row_d = nc.dram_tensor("dis2row", [128, NPAD], dt.float16, kind="ExternalInput")
    disrow_d = nc.dram_tensor("disrow", [128, NPAD], dt.float16, kind="ExternalInput")
    if not B1ZERO:
        crow_d = nc.dram_tensor("crow", [128, NPAD], dt.float32, kind="ExternalInput")
    out_d = nc.dram_tensor("out", [128, NW * FOUT], dt.float32, kind="ExternalOutput")

    t1_local = nc.dram_tensor("t1_local", [NPAD, FHID], dt.float16)
    t1_full = nc.dram_tensor("t1_full", [NFULL, FHID], dt.float16, addr_space="Shared")
    t2_local = nc.dram_tensor("t2_local", [NPAD, FHID], dt.float16)
    t2_full = nc.dram_tensor("t2_full", [NFULL, FHID], dt.float16, addr_space="Shared")

    NMM = -(-NPAD // 512)  # 13 matmul groups of 512 nodes

    with tile.TileContext(nc) as tc:
        with tc.tile_pool(name="consts", bufs=1) as cp:
            w1_t = cp.tile([128, 2, FHID], dt.float16, tag="w1")
            nc.sync.dma_start(w1_t[:], W1_d[:, :].rearrange("p (k f) -> p k f", k=2))
            w2_t = cp.tile([FHID, FOUT], dt.float16, tag="w2")
            nc.sync.dma_start(w2_t[:], W2_d[:, :])
            b2v_t = cp.tile([FOUT, 1], dt.float32, tag="b2v")
            nc.sync.dma_start(b2v_t[:], b2v_d[:, :])
            dis2row_t = cp.tile([128, NPAD], dt.float16, tag="dis2row")
            nc.sync.dma_start(dis2row_t[:], dis2row_d[:, :])
            disrow_t = cp.tile([128, NPAD], dt.float16, tag="disrow")
            nc.sync.dma_start(disrow_t[:], disrow_d[:, :])
            gidx_t = cp.tile([128, TOKTOT // 16], dt.int16, tag="gidx")
            nc.sync.dma_start(gidx_t[:], gidx_d[:, :])
            if not B1ZERO:
                crow_t = cp.tile([128, NPAD], dt.float32, tag="crow")
                nc.sync.dma_start(crow_t[:], crow_d[:, :])

            nidx_regs = {cnt: nc.gpsimd.to_reg(cnt) for cnt in chunk_counts}

            def agg_layer(lname, src_full, red_all, redg, tokbuf):
                """Gather+reduce all groups of one layer into red_all."""
                base_lo = src_full[0:LOROWS, :]
                base_hi = src_full[LOROWS:NFULL, :]
                goff = 0
                qn = 0
                if SKIP_GATHER:
                    nc.vector.memset(tokbuf[:], 0.0)
                for (w0, gw, L, H) in groups:
                    for s, (pad, base) in enumerate(((L, base_lo), (H, base_hi))):
                        buf = qn % 2 if DBLBUF else 0
                        T = pad * gw * 128
                        o = 0
                        while o < T and not SKIP_GATHER:
                            cnt = min(MAXIDX, T - o)
                            c0 = (goff + o) // 16
                            nc.gpsimd.dma_gather(
                                tokbuf[:, buf:buf + 1, o:o + cnt], base,
                                gidx_t[:, c0:c0 + cnt // 16],
                                num_idxs=cnt, num_idxs_reg=nidx_regs[cnt],
                                elem_size=FHID, single_packet=False,
                                transpose=True, queue_num=qn % NQUEUES)
                            o += cnt
                        qn += 1
                        red_out = (red_all if s == 0 else redg)
                        col0 = (w0 * 128 if s == 0 else 0)
                        nc.vector.tensor_reduce(
                            red_out[:, col0:col0 + gw * 128]
                            .rearrange("f (b p) -> f b p", b=gw),
                            tokbuf[:, buf, 0:T]
                            .rearrange("f (b k p) -> f b p k", b=gw, p=128),
                            AXL.X, ALU.add)
                        goff += T
                    nc.vector.tensor_tensor(
                        red_all[:, w0 * 128:(w0 + gw) * 128],
                        red_all[:, w0 * 128:(w0 + gw) * 128],
                        redg[:, 0:gw * 128], ALU.add)

            for _rep in range(REPEAT):
                # ---- phase B: h1T = W1^T @ (dis*x)^T, f-major ----
                with (
                    tc.tile_pool(name="phaseB", bufs=1) as pb,
                    tc.tile_pool(name="psB", bufs=1, space="PSUM") as ppb,
                ):
                    xT_t = pb.tile([128, 2, NPAD], dt.float16, tag="xT")
                    nc.sync.dma_start(
                        xT_t[:], xT_d[:, :].rearrange("p (k n) -> p k n", k=2))
                    h1T = pb.tile([128, NPAD], dt.float16, tag="h1T")
                    psB = ppb.tile([128, 8, 512], dt.float32, tag="pB")
                    for gi in range(NMM):
                        n0 = gi * 512
                        cols = min(512, NPAD - n0)
                        sl = psB[:, gi % 8, 0:cols]
                        for kc in range(2):
                            nc.tensor.matmul(
                                sl, w1_t[:, kc, :],
                                xT_t[:, kc, n0:n0 + cols],
                                start=(kc == 0), stop=(kc == 1))
                        if gi % 8 == 7:
                            nc.vector.tensor_copy(
                                h1T[:, (gi - 7) * 512:(gi + 1) * 512], psB[:])
                        elif gi == NMM - 1:
                            nfull = gi % 8   # full 512-col banks in tail
                            if nfull:
                                nc.vector.tensor_copy(
                                    h1T[:, 4096:4096 + nfull * 512],
                                    psB[:, 0:nfull, :])
                            nc.vector.tensor_copy(
                                h1T[:, 4096 + nfull * 512:NPAD],
                                psB[:, nfull:nfull + 1, 0:cols])
                    stage = pb.tile([128, NW, FHID], dt.float16, tag="stageB")
                    nc.sync.dma_start(stage[:], h1T[:], transpose=True)
                    nc.sync.dma_start(
                        t1_local[:, :].rearrange("(p s) f -> p s f", p=128),
                        stage[:])

                if not SKIP_AG:
                    nc.gpsimd.collective_compute(
                        "AllGather", mybir.AluOpType.bypass,
                        replica_groups=[list(range(NCORES))],
                        ins=[t1_local[:, :]], outs=[t1_full[:, :]],
                    )

                if PHASES in ("B", "B0"):
                    with tc.tile_pool(name="dummy", bufs=1) as dp:
                        ot = dp.tile([128, NW * FOUT], dt.float32, tag="o")
                        nc.vector.memset(ot[:], 0.0)
                        nc.sync.dma_start(out_d[:, :], ot[:])
                    continue

                # ---- L1 aggregation ----
                with tc.tile_pool(name="L1", bufs=1) as l1:
                    tokbuf = l1.tile([128, 2 if DBLBUF else 1, TOKCAP],
                                     dt.float16, tag="tok1")
                    red_all = l1.tile([128, NPAD], dt.float32, tag="red1")
                    redg = l1.tile([128, GWCAP * 128], dt.float32, tag="redg1")
                    o1T = l1.tile([128, NPAD], dt.float16, tag="o1T")
                    agg_layer("L1", t1_full, red_all, redg, tokbuf)
                    if not B1ZERO:
                        nc.vector.tensor_tensor(
                            red_all[:], red_all[:], crow_t[:], ALU.add)
                    nc.vector.scalar_tensor_tensor(
                        o1T[:], red_all[:], 0.0, dis2row_t[:],
                        ALU.max, ALU.mult)
                    stage1 = l1.tile([128, NW, FHID], dt.float16, tag="stage1")
                    nc.sync.dma_start(stage1[:], o1T[:], transpose=True)
                    nc.sync.dma_start(
                        t2_local[:, :].rearrange("(p s) f -> p s f", p=128),
                        stage1[:])

                if PHASES == "B1":
                    with tc.tile_pool(name="dummy2", bufs=1) as dp:
                        ot = dp.tile([128, NW * FOUT], dt.float32, tag="o")
                        nc.vector.memset(ot[:], 0.0)
                        nc.sync.dma_start(out_d[:, :], ot[:])
                    continue

                if not SKIP_AG:
                    nc.gpsimd.collective_compute(
                        "AllGather", mybir.AluOpType.bypass,
                        replica_groups=[list(range(NCORES))],
                        ins=[t2_local[:, :]], outs=[t2_full[:, :]],
                    )

                # ---- L2: aggregate o1, then @W2 + b2 ----
                with (
                    tc.tile_pool(name="L2", bufs=1) as l2,
                    tc.tile_pool(name="ps2p", bufs=1, space="PSUM") as pp,
                ):
                    tokbuf = l2.tile([128, 2 if DBLBUF else 1, TOKCAP],
                                     dt.float16, tag="tok2")
                    red_all = l2.tile([128, NPAD], dt.float32, tag="red2")
                    redg = l2.tile([128, GWCAP * 128], dt.float32, tag="redg2")
                    r2T = l2.tile([128, NPAD], dt.float16, tag="r2T")
                    agg_layer("L2", t2_full, red_all, redg, tokbuf)
                    nc.vector.tensor_tensor(
                        r2T[:], red_all[:], disrow_t[:], ALU.mult)
                    h2T = l2.tile([128, NPAD], dt.float16, tag="h2T")
                    ps2 = pp.tile([128, 8, 512], dt.float32, tag="p2")
                    for gi in range(NMM):
                        n0 = gi * 512
                        cols = min(512, NPAD - n0)
                        nc.tensor.matmul(
                            ps2[0:FOUT, gi % 8, 0:cols], w2_t[:],
                            r2T[:, n0:n0 + cols], start=True, stop=True)
                        if gi % 8 == 7:
                            nc.vector.tensor_scalar(
                                h2T[0:FOUT, (gi - 7) * 512:(gi + 1) * 512],
                                ps2[0:FOUT, :, :], b2v_t[:, 0:1], None, ALU.add)
                        elif gi == NMM - 1:
                            nfull = gi % 8
                            if nfull:
                                nc.vector.tensor_scalar(
                                    h2T[0:FOUT, 4096:4096 + nfull * 512],
                                    ps2[0:FOUT, 0:nfull, :],
                                    b2v_t[:, 0:1], None, ALU.add)
                            nc.vector.tensor_scalar(
                                h2T[0:FOUT, 4096 + nfull * 512:NPAD],
                                ps2[0:FOUT, nfull:nfull + 1, 0:cols],
                                b2v_t[:, 0:1], None, ALU.add)
                    stage2 = l2.tile([128, NW, FOUT], dt.float16, tag="stage2")
                    nc.sync.dma_start(stage2[:], h2T[0:FOUT, :], transpose=True)
                    outst = l2.tile([128, NW, FOUT], dt.float32, tag="outst")
                    nc.vector.tensor_copy(outst[:], stage2[:])
                    nc.sync.dma_start(out_d[:, :],
                                      outst[:].rearrange("p s f -> p (s f)"))

    nc.compile()
    return nc


def kernel(x, edge_index, W1, b1, W2, b2):
    global LAST_RESULTS
    from concourse.bass_utils import run_bass_kernel_spmd

    in_maps, Kinfo = _host_prep(x, edge_index, W1, b1, W2, b2)
    key = Kinfo[:4]
    if key not in _CACHE:
        _CACHE[key] = _build(Kinfo)
    nc = _CACHE[key]

    res = run_bass_kernel_spmd(nc, in_maps, list(range(NCORES)))
    LAST_RESULTS = res

    core_of, n_of = Kinfo[4], Kinfo[5]
    out = np.empty((N, FOUT), np.float32)
    for c in range(NCORES):
        mine = np.where(core_of == c)[0]
        arr = res.results[c]["out"].reshape(128, NW, FOUT)
        out[mine] = arr[n_of[mine] % 128, n_of[mine] // 128]
    return out
